# revision 6
# baseline (speedup 1.0000x reference)
"""Contextual loss (CX) kernel for Trainium2, 8 NeuronCores.

Sharding: data-parallel over (image, row-half): core c handles image c//2,
pred-rows [ (c%2)*2048, (c%2+1)*2048 ) of the 4096x4096 contextual matrix.

Math (per core, rows i of its half, columns j over all HW):
    pc_i   = p_i - mu          (mu = target mean feature; fp8 quantized)
    that_j = (t_j - mu)/||t_j - mu||                      (fp8 quantized)
    raw_ij = <pc_i, that_j>    (fp8 DoubleRow matmul, fp32 PSUM)
    s_ij   = raw_ij / n_i,  n_i = ||pc_i||  (from the quantized pc)
    e_ij   = exp(b_i (s_ij - smax_i)) = exp(scale_i*raw_ij + bias_i)
             with b_i = 1/(1+EPS-smax_i), scale_i = b_i/n_i,
             bias_i = -scale_i*rawmax_i
    rs_i   = sum_j e_ij    (ACT accumulate)
    M_j    = max(M_j, e_ij / rs_i)   (partition-wise partial column max)
Host folds partitions + row-halves, means over j, -log, means over N.
Identical to the reference up to per-row constants that cancel in the
row normalization.

Engine split per 128-row block (PSUM held as four 2-bank pair tiles):
  PE   16 fp8 DoubleRow matmuls
  DMA  pair 0 eviction (gpsimd SWDGE with f32->f16 cast)
  ACT  pair 1 eviction copy; exp with rowsum-accumulate
  DVE  pairs 2,3 fused eviction + rowmax; fp16 tree for pairs 0,1;
       per-row scalars; column-max update as fast fp16 mul/max chunks
       against a rinv-replicated [128,512] tile
"""

import os
import numpy as np
from contextlib import ExitStack

import concourse.bass as bass
import concourse.bacc as bacc
import concourse.mybir as mybir
import concourse.tile as tile
from concourse.bass_utils import run_bass_kernel_spmd

F32 = mybir.dt.float32
F16 = mybir.dt.float16
F8 = mybir.dt.float8e4
AX = mybir.AxisListType.X
ALU = mybir.AluOpType
ACTF = mybir.ActivationFunctionType
DR = mybir.MatmulPerfMode.DoubleRow

N_IMG, C, H, W = 4, 512, 64, 64
HW = H * W              # 4096
R = HW // 2             # 2048 rows per core
KB = C // 128           # 4 contraction blocks
NPAIR = KB // 2         # 2 DoubleRow pairs
NB = R // 16 // 8       # 16 row blocks per core
CH = 512                # column chunk (one PSUM bank)
NCH = HW // CH          # 8 chunks
PW = 2 * CH             # PSUM pair-tile width
EPS = 1e-5
DMAEV = os.environ.get("CX_DMAEV", "1") == "1"


def _build_nc():
    nc = bacc.Bacc("TRN2", target_bir_lowering=False, debug=False, num_devices=8)
    t_dram = nc.dram_tensor("t", [C, HW], F32, kind="ExternalInput").ap()
    p_dram = nc.dram_tensor("p", [C, R], F32, kind="ExternalInput").ap()
    m_dram = nc.dram_tensor("m_out", [128, HW], F16, kind="ExternalOutput").ap()

    with tile.TileContext(nc) as tc, ExitStack() as ctx:
        const = ctx.enter_context(tc.tile_pool(name="const", bufs=1))
        ones16 = const.tile([128, 128], F16, tag="ones", name="ones16")
        nc.vector.memset(ones16[:], 1.0)
        ones512 = const.tile([128, CH], F16, tag="ones512", name="ones512")
        nc.vector.memset(ones512[:], 1.0)
        # fp8 operands in DoubleRow pair-interleaved layout: pair p holds
        # contraction blocks 2p (dim1=0) and 2p+1 (dim1=1)
        that8 = [const.tile([128, 2, HW], F8, tag=f"that{p}", name=f"that{p}")
                 for p in range(NPAIR)]
        pc8 = [const.tile([128, 2, R], F8, tag=f"pc{p}", name=f"pc{p}")
               for p in range(NPAIR)]
        rinvn = const.tile([128, NB], F32, tag="rinvn", name="rinvn")
        obs = [const.tile([1, 1], F32, tag=f"obs{i}", name=f"obs{i}")
               for i in range(NB)]

        # ---------------- preprocessing ----------------
        with (
            tc.tile_pool(name="raw", bufs=1) as raw,
            tc.tile_pool(name="sqp", bufs=2) as sqp,
        ):
            traw = [raw.tile([128, HW], F32, tag=f"traw{k}", name=f"traw{k}") for k in range(KB)]
            praw = [raw.tile([128, R], F32, tag=f"praw{k}", name=f"praw{k}") for k in range(KB)]
            t16 = [raw.tile([128, HW], F16, tag=f"t16_{k}", name=f"t16_{k}") for k in range(KB)]
            tsum = [raw.tile([128, 1], F32, tag=f"tsum{k}", name=f"tsum{k}") for k in range(KB)]
            negmu = [raw.tile([128, 1], F32, tag=f"negmu{k}", name=f"negmu{k}") for k in range(KB)]
            psq = [raw.tile([128, R], F16, tag=f"psq{k}", name=f"psq{k}") for k in range(KB)]
            lnm = raw.tile([128, HW], F16, tag="lnm", name="lnm")
            invm = raw.tile([128, HW], F16, tag="invm", name="invm")
            nsq_sb = raw.tile([128, NB], F32, tag="nsq_sb", name="nsq_sb")
            lnn = raw.tile([128, NB], F32, tag="lnn", name="lnn")

            for k in range(KB):
                nc.sync.dma_start(traw[k][:], t_dram[k * 128:(k + 1) * 128, :])
            for k in range(KB):
                nc.sync.dma_start(praw[k][:], p_dram[k * 128:(k + 1) * 128, :])

            # target per-channel mean (ACT accumulate) + f16 cast
            for k in range(KB):
                nc.scalar.activation(t16[k][:], traw[k][:], ACTF.Identity,
                                     accum_out=tsum[k][:])
            for k in range(KB):
                nc.vector.tensor_scalar(negmu[k][:], tsum[k][:], -1.0 / HW, None, ALU.mult)

            # pred: center -> fp8; square (from the quantized pc for
            # consistency); transposed column-sums -> nsq [128, NB]
            for k in range(KB):
                pslice = pc8[k // 2][:, k % 2, :]
                nc.vector.tensor_scalar(pslice, praw[k][:], negmu[k][:], None, ALU.add)
                nc.vector.tensor_mul(psq[k][:], pslice, pslice)
            with tc.tile_pool(name="nsqps", bufs=1, space="PSUM") as nsqps:
                nsq_ps = nsqps.tile([128, NB], F32, tag="nsq", name="nsq_ps")
                for ib in range(NB):
                    for k in range(KB):
                        nc.tensor.matmul(
                            nsq_ps[:, ib:ib + 1],
                            psq[k][:, ib * 128:(ib + 1) * 128],
                            ones16[:, 0:1],
                            start=(k == 0),
                            stop=(k == KB - 1),
                        )
                nc.vector.tensor_scalar(nsq_sb[:], nsq_ps[:], 1.0, None, ALU.mult)

            # target: center in place (f16), square, column-sums -> msq
            with tc.tile_pool(name="msqps", bufs=1, space="PSUM") as msqps:
                msq = msqps.tile([128, HW], F32, tag="msq", name="msq")
                for k in range(KB):
                    nc.vector.tensor_scalar(t16[k][:], t16[k][:], negmu[k][:], None, ALU.add)
                    sq = sqp.tile([128, HW], F16, tag="sq", name="sq")
                    nc.vector.tensor_mul(sq[:], t16[k][:], t16[k][:])
                    for j in range(NCH):
                        nc.tensor.matmul(
                            msq[:, j * CH:(j + 1) * CH],
                            ones16[:],
                            sq[:, j * CH:(j + 1) * CH],
                            start=(k == 0),
                            stop=(k == KB - 1),
                        )

                # x^-0.5 = exp(-0.5 ln x); avoids banned Rsqrt + DVE reciprocal
                nc.scalar.activation(lnn[:], nsq_sb[:], ACTF.Ln)
                nc.scalar.activation(lnm[:], msq[:], ACTF.Ln)
            nc.scalar.activation(invm[:], lnm[:], ACTF.Exp, scale=-0.5)
            nc.scalar.activation(rinvn[:], lnn[:], ACTF.Exp, scale=-0.5)

            # that8 = (t - mu) * invm -> fp8
            for k in range(KB):
                nc.vector.tensor_mul(that8[k // 2][:, k % 2, :], t16[k][:], invm[:])

        # ---------------- main loop ----------------
        main = ctx.enter_context(tc.tile_pool(name="main", bufs=2))
        stats = ctx.enter_context(tc.tile_pool(name="stats", bufs=2))
        mainps = ctx.enter_context(tc.tile_pool(name="mainps", bufs=4, space="PSUM"))
        mACC = main.tile([128, HW], F16, tag="mACC", bufs=1, name="mACC")
        nc.vector.memset(mACC[:], 0.0)

        reps = int(os.environ.get("CX_REPS", "1"))
        for ib in [i for _ in range(reps) for i in range(NB)]:
            s16 = main.tile([128, HW], F16, tag="s", name="s16")
            e16 = main.tile([128, HW], F16, tag="e", name="e16")
            cmax = stats.tile([128, 4], F32, tag="cmax", name="cmax")
            tr = stats.tile([128, PW], F16, tag="tr", name="tr")
            rawmax = stats.tile([128, 1], F32, tag="rawmax", name="rawmax")
            smax = stats.tile([128, 1], F32, tag="smax", name="smax")
            t1 = stats.tile([128, 1], F32, tag="t1", name="t1")
            bb = stats.tile([128, 1], F32, tag="bb", name="bb")
            scaleP = stats.tile([128, 1], F32, tag="scaleP", name="scaleP")
            biasP = stats.tile([128, 1], F32, tag="biasP", name="biasP")
            rs = stats.tile([128, 1], F32, tag="rs", name="rs")
            rinv = stats.tile([128, 1], F32, tag="rinv", name="rinv")
            rinv512 = stats.tile([128, CH], F16, tag="rinv512", name="rinv512")
            qv = rinvn[:, ib:ib + 1]

            # four 2-bank PSUM pair tiles; chunks 2p, 2p+1 live in pair p
            pss = [mainps.tile([128, PW], F32, tag="ps", name="ps") for _ in range(4)]
            for jc in range(NCH):
                pt = pss[jc // 2]
                out = pt[:, (jc % 2) * CH:(jc % 2 + 1) * CH]
                for pair in range(NPAIR):
                    nc.tensor.matmul(
                        out,
                        pc8[pair][:, :, ib * 128:(ib + 1) * 128],
                        that8[pair][:, :, jc * CH:(jc + 1) * CH],
                        start=(pair == 0),
                        stop=(pair == NPAIR - 1),
                        perf_mode=DR,
                    )
                if jc % 2 == 1:
                    p = jc // 2
                    dst = s16[:, p * PW:(p + 1) * PW]
                    if p == 0:
                        # split: first chunk DVE-fused w/ rowmax, second ACT
                        nc.vector.tensor_scalar(
                            s16[:, 0:CH], pt[:, 0:CH], 1.0, None,
                            ALU.mult, ALU.max, accum_out=cmax[:, 0:1],
                        )
                        nc.scalar.copy(s16[:, CH:PW], pt[:, CH:PW])
                    elif p == 1:
                        nc.scalar.copy(dst, pt[:])
                    else:
                        # fused PSUM->SBUF copy + row-max accumulation
                        nc.vector.tensor_scalar(
                            dst, pt[:], 1.0, None, ALU.mult, ALU.max,
                            accum_out=cmax[:, p:p + 1],
                        )

            # row-max of the ACT-copied cols [CH:2*PW] via fast fp16 tree
            nc.vector.tensor_max(tr[:, :CH], s16[:, PW:PW + CH], s16[:, PW + CH:2 * PW])
            nc.vector.tensor_max(tr[:, :CH], tr[:, :CH], s16[:, CH:PW])
            nc.vector.reduce_max(cmax[:, 1:2], tr[:, :CH], axis=AX)
            nc.vector.reduce_max(rawmax[:], cmax[:, 0:4], axis=AX)

            # b=1/(1+EPS-rawmax*q); scale=b*q; bias=-scale*rawmax
            nc.vector.tensor_mul(smax[:], rawmax[:], qv)
            nc.vector.tensor_scalar(t1[:], smax[:], -1.0, 1.0 + EPS, ALU.mult, ALU.add)
            nc.vector.reciprocal(bb[:], t1[:])
            nc.vector.tensor_mul(scaleP[:], bb[:], qv)
            nc.vector.scalar_tensor_tensor(
                biasP[:], scaleP[:], -1.0, rawmax[:], ALU.mult, ALU.mult
            )

            nc.scalar.activation(
                e16[:], s16[:], ACTF.Exp, bias=biasP[:], scale=scaleP[:],
                accum_out=rs[:],
            )
            nc.vector.reciprocal(rinv[:], rs[:])
            # M = max(M, e*rinv) via fast fp16 tensor_tensor chunks
            nc.vector.tensor_scalar(rinv512[:], ones512[:], rinv[:], None, ALU.mult)
            for q in range(NCH):
                ec = e16[:, q * CH:(q + 1) * CH]
                nc.vector.tensor_mul(ec, ec, rinv512[:])
                nc.vector.tensor_max(mACC[:, q * CH:(q + 1) * CH],
                                     mACC[:, q * CH:(q + 1) * CH], ec)

        nc.sync.dma_start(m_dram[:, :], mACC[:])
    nc.compile()
    return nc


_NC_CACHE = {}


def _get_nc():
    if "nc" not in _NC_CACHE:
        _NC_CACHE["nc"] = _build_nc()
    return _NC_CACHE["nc"]


def kernel(pred, target, _trace=False):
    pred = np.asarray(pred, dtype=np.float32).reshape(N_IMG, C, HW)
    target = np.asarray(target, dtype=np.float32).reshape(N_IMG, C, HW)
    nc = _get_nc()
    in_maps = []
    for core in range(8):
        img, half = divmod(core, 2)
        in_maps.append({
            "t": np.ascontiguousarray(target[img]),
            "p": np.ascontiguousarray(pred[img, :, half * R:(half + 1) * R]),
        })
    res = run_bass_kernel_spmd(nc, in_maps, list(range(8)), trace=_trace)
    losses = []
    for img in range(N_IMG):
        m0 = res.results[2 * img]["m_out"].astype(np.float32).max(axis=0)
        m1 = res.results[2 * img + 1]["m_out"].astype(np.float32).max(axis=0)
        cx = np.maximum(m0, m1).mean()
        losses.append(-np.log(cx + EPS))
    out = np.float32(np.mean(losses))
    if _trace:
        return out, res
    return out


# revision 12
# speedup vs baseline: 1.0027x; 1.0027x over previous
"""Contextual loss (CX) kernel for Trainium2, 8 NeuronCores.

Sharding: data-parallel over (image, row-half): core c handles image c//2,
pred-rows [ (c%2)*2048, (c%2+1)*2048 ) of the 4096x4096 contextual matrix.

Math (per core, rows i of its half, columns j over all HW):
    pc_i   = p_i - mu          (mu = target mean feature; fp8 quantized)
    that_j = (t_j - mu)/||t_j - mu||                      (fp8 quantized)
    raw_ij = <pc_i, that_j>    (fp8 DoubleRow matmul, fp32 PSUM)
    s_ij   = raw_ij / n_i,  n_i = ||pc_i||  (from the quantized pc)
    e_ij   = exp(b_i (s_ij - smax_i)) = exp(scale_i*raw_ij + bias_i)
             with b_i = 1/(1+EPS-smax_i), scale_i = b_i/n_i,
             bias_i = -scale_i*rawmax_i
    rs_i   = sum_j e_ij    (ACT accumulate)
    M_j    = max(M_j, e_ij / rs_i)   (partition-wise partial column max)
Host folds partitions + row-halves, means over j, -log, means over N.
Identical to the reference up to per-row constants that cancel in the
row normalization.

Engine split per 128-row block (PSUM held as four 2-bank pair tiles):
  PE   16 fp8 DoubleRow matmuls
  DMA  pair 0 eviction (gpsimd SWDGE with f32->f16 cast)
  ACT  pair 1 eviction copy; exp with rowsum-accumulate
  DVE  pairs 2,3 fused eviction + rowmax; fp16 tree for pairs 0,1;
       per-row scalars; column-max update as fast fp16 mul/max chunks
       against a rinv-replicated [128,512] tile
"""

import os
import numpy as np
from contextlib import ExitStack

import concourse.bass as bass
import concourse.bacc as bacc
import concourse.mybir as mybir
import concourse.tile as tile
from concourse.bass_utils import run_bass_kernel_spmd

F32 = mybir.dt.float32
F16 = mybir.dt.float16
F8 = mybir.dt.float8e4
AX = mybir.AxisListType.X
ALU = mybir.AluOpType
ACTF = mybir.ActivationFunctionType
DR = mybir.MatmulPerfMode.DoubleRow

N_IMG, C, H, W = 4, 512, 64, 64
HW = H * W              # 4096
R = HW // 2             # 2048 rows per core
KB = C // 128           # 4 contraction blocks
NPAIR = KB // 2         # 2 DoubleRow pairs
NB = R // 16 // 8       # 16 row blocks per core
CH = 512                # column chunk (one PSUM bank)
NCH = HW // CH          # 8 chunks
PW = 2 * CH             # PSUM pair-tile width
EPS = 1e-5


def _build_nc():
    nc = bacc.Bacc("TRN2", target_bir_lowering=False, debug=False, num_devices=8)
    t_dram = nc.dram_tensor("t", [C, HW], F32, kind="ExternalInput").ap()
    p_dram = nc.dram_tensor("p", [C, R], F32, kind="ExternalInput").ap()
    m_dram = nc.dram_tensor("m_out", [128, HW], F16, kind="ExternalOutput").ap()

    with tile.TileContext(nc) as tc, ExitStack() as ctx:
        const = ctx.enter_context(tc.tile_pool(name="const", bufs=1))
        ones16 = const.tile([128, 128], F16, tag="ones", name="ones16")
        nc.vector.memset(ones16[:], 1.0)
        ones512 = const.tile([128, CH], F16, tag="ones512", name="ones512")
        nc.vector.memset(ones512[:], 1.0)
        # fp8 operands in DoubleRow pair-interleaved layout: pair p holds
        # contraction blocks 2p (dim1=0) and 2p+1 (dim1=1)
        that8 = [const.tile([128, 2, HW], F8, tag=f"that{p}", name=f"that{p}")
                 for p in range(NPAIR)]
        pc8 = [const.tile([128, 2, R], F8, tag=f"pc{p}", name=f"pc{p}")
               for p in range(NPAIR)]
        rinvn = const.tile([128, NB], F32, tag="rinvn", name="rinvn")

        # ---------------- preprocessing ----------------
        with (
            tc.tile_pool(name="raw", bufs=1) as raw,
            tc.tile_pool(name="sqp", bufs=2) as sqp,
        ):
            traw = [raw.tile([128, HW], F32, tag=f"traw{k}", name=f"traw{k}") for k in range(KB)]
            praw = [raw.tile([128, R], F32, tag=f"praw{k}", name=f"praw{k}") for k in range(KB)]
            t16 = [raw.tile([128, HW], F16, tag=f"t16_{k}", name=f"t16_{k}") for k in range(KB)]
            tsum = [raw.tile([128, 1], F32, tag=f"tsum{k}", name=f"tsum{k}") for k in range(KB)]
            negmu = [raw.tile([128, 1], F32, tag=f"negmu{k}", name=f"negmu{k}") for k in range(KB)]
            psq = [raw.tile([128, R], F16, tag=f"psq{k}", name=f"psq{k}") for k in range(KB)]
            lnm = raw.tile([128, HW], F16, tag="lnm", name="lnm")
            invm = raw.tile([128, HW], F16, tag="invm", name="invm")
            nsq_sb = raw.tile([128, NB], F32, tag="nsq_sb", name="nsq_sb")
            lnn = raw.tile([128, NB], F32, tag="lnn", name="lnn")

            for k in range(KB):
                nc.sync.dma_start(traw[k][:], t_dram[k * 128:(k + 1) * 128, :])
            for k in range(KB):
                nc.sync.dma_start(praw[k][:], p_dram[k * 128:(k + 1) * 128, :])

            # target per-channel mean (ACT accumulate) + f16 cast
            for k in range(KB):
                nc.scalar.activation(t16[k][:], traw[k][:], ACTF.Identity,
                                     accum_out=tsum[k][:])
            for k in range(KB):
                nc.vector.tensor_scalar(negmu[k][:], tsum[k][:], -1.0 / HW, None, ALU.mult)

            # pred: center -> fp8 (ACT, fused bias); square from the
            # quantized pc for consistency; transposed column-sums -> nsq
            for k in range(KB):
                pslice = pc8[k // 2][:, k % 2, :]
                nc.scalar.activation(pslice, praw[k][:], ACTF.Identity,
                                     bias=negmu[k][:])
                nc.vector.tensor_mul(psq[k][:], pslice, pslice)
            with tc.tile_pool(name="nsqps", bufs=1, space="PSUM") as nsqps:
                nsq_ps = nsqps.tile([128, NB], F32, tag="nsq", name="nsq_ps")
                for ib in range(NB):
                    for k in range(KB):
                        nc.tensor.matmul(
                            nsq_ps[:, ib:ib + 1],
                            psq[k][:, ib * 128:(ib + 1) * 128],
                            ones16[:, 0:1],
                            start=(k == 0),
                            stop=(k == KB - 1),
                        )
                nc.vector.tensor_scalar(nsq_sb[:], nsq_ps[:], 1.0, None, ALU.mult)

            # target: center in place (f16), square, column-sums -> msq
            with tc.tile_pool(name="msqps", bufs=1, space="PSUM") as msqps:
                msq = msqps.tile([128, HW], F32, tag="msq", name="msq")
                for k in range(KB):
                    nc.vector.tensor_scalar(t16[k][:], t16[k][:], negmu[k][:], None, ALU.add)
                    sq = sqp.tile([128, HW], F16, tag="sq", name="sq")
                    nc.vector.tensor_mul(sq[:], t16[k][:], t16[k][:])
                    for j in range(NCH):
                        nc.tensor.matmul(
                            msq[:, j * CH:(j + 1) * CH],
                            ones16[:],
                            sq[:, j * CH:(j + 1) * CH],
                            start=(k == 0),
                            stop=(k == KB - 1),
                        )

                # x^-0.5 = exp(-0.5 ln x); avoids banned Rsqrt + DVE reciprocal
                nc.scalar.activation(lnn[:], nsq_sb[:], ACTF.Ln)
                nc.scalar.activation(lnm[:], msq[:], ACTF.Ln)
            nc.scalar.activation(invm[:], lnm[:], ACTF.Exp, scale=-0.5)
            nc.scalar.activation(rinvn[:], lnn[:], ACTF.Exp, scale=-0.5)

            # that8 = (t - mu) * invm -> fp8
            for k in range(KB):
                nc.vector.tensor_mul(that8[k // 2][:, k % 2, :], t16[k][:], invm[:])

        # ---------------- main loop ----------------
        main = ctx.enter_context(tc.tile_pool(name="main", bufs=2))
        stats = ctx.enter_context(tc.tile_pool(name="stats", bufs=2))
        mainps = ctx.enter_context(tc.tile_pool(name="mainps", bufs=4, space="PSUM"))
        # ping-pong column-max accumulators: DVE fast-mode tensor ops need
        # out to not alias either input
        macc = [main.tile([128, HW], F16, tag=f"mACC{i}", bufs=1, name=f"mACC{i}")
                for i in range(2)]
        nc.vector.memset(macc[0][:], 0.0)

        reps = int(os.environ.get("CX_REPS", "1"))
        ib_list = [i for _ in range(reps) for i in range(NB)]
        for it, ib in enumerate(ib_list):
            s16 = main.tile([128, HW], F16, tag="s", name="s16")
            e16 = main.tile([128, HW], F16, tag="e", name="e16")
            em = main.tile([128, HW], F16, tag="em", name="em")
            cmax = stats.tile([128, 4], F32, tag="cmax", name="cmax")
            tra = stats.tile([128, CH], F16, tag="tra", name="tra")
            trb = stats.tile([128, CH], F16, tag="trb", name="trb")
            rawmax = stats.tile([128, 1], F32, tag="rawmax", name="rawmax")
            smax = stats.tile([128, 1], F32, tag="smax", name="smax")
            t1 = stats.tile([128, 1], F32, tag="t1", name="t1")
            bb = stats.tile([128, 1], F32, tag="bb", name="bb")
            scaleP = stats.tile([128, 1], F32, tag="scaleP", name="scaleP")
            biasP = stats.tile([128, 1], F32, tag="biasP", name="biasP")
            rs = stats.tile([128, 1], F32, tag="rs", name="rs")
            rinv = stats.tile([128, 1], F32, tag="rinv", name="rinv")
            rinv512 = stats.tile([128, CH], F16, tag="rinv512", name="rinv512")
            qv = rinvn[:, ib:ib + 1]

            # four 2-bank PSUM pair tiles; chunks 2p, 2p+1 live in pair p
            pss = [mainps.tile([128, PW], F32, tag="ps", name="ps") for _ in range(4)]
            for jc in range(NCH):
                pt = pss[jc // 2]
                out = pt[:, (jc % 2) * CH:(jc % 2 + 1) * CH]
                for pair in range(NPAIR):
                    nc.tensor.matmul(
                        out,
                        pc8[pair][:, :, ib * 128:(ib + 1) * 128],
                        that8[pair][:, :, jc * CH:(jc + 1) * CH],
                        start=(pair == 0),
                        stop=(pair == NPAIR - 1),
                        perf_mode=DR,
                    )
                if jc % 2 == 1:
                    p = jc // 2
                    dst = s16[:, p * PW:(p + 1) * PW]
                    if p == 0:
                        # split: first chunk DVE-fused w/ rowmax, second ACT
                        nc.vector.tensor_scalar(
                            s16[:, 0:CH], pt[:, 0:CH], 1.0, None,
                            ALU.mult, ALU.max, accum_out=cmax[:, 0:1],
                        )
                        nc.scalar.copy(s16[:, CH:PW], pt[:, CH:PW])
                    elif p == 1:
                        nc.scalar.copy(dst, pt[:])
                    else:
                        # fused PSUM->SBUF copy + row-max accumulation
                        nc.vector.tensor_scalar(
                            dst, pt[:], 1.0, None, ALU.mult, ALU.max,
                            accum_out=cmax[:, p:p + 1],
                        )

            # row-max of the ACT-copied cols [CH:2*PW] via fast fp16 tree
            nc.vector.tensor_max(tra[:], s16[:, PW:PW + CH], s16[:, PW + CH:2 * PW])
            nc.vector.tensor_max(trb[:], tra[:], s16[:, CH:PW])
            nc.vector.reduce_max(cmax[:, 1:2], trb[:], axis=AX)
            nc.vector.reduce_max(rawmax[:], cmax[:, 0:4], axis=AX)

            # b=1/(1+EPS-rawmax*q); scale=b*q; bias=-scale*rawmax
            nc.vector.tensor_mul(smax[:], rawmax[:], qv)
            nc.vector.tensor_scalar(t1[:], smax[:], -1.0, 1.0 + EPS, ALU.mult, ALU.add)
            nc.vector.reciprocal(bb[:], t1[:])
            nc.vector.tensor_mul(scaleP[:], bb[:], qv)
            nc.vector.scalar_tensor_tensor(
                biasP[:], scaleP[:], -1.0, rawmax[:], ALU.mult, ALU.mult
            )

            nc.scalar.activation(
                e16[:], s16[:], ACTF.Exp, bias=biasP[:], scale=scaleP[:],
                accum_out=rs[:],
            )
            nc.vector.reciprocal(rinv[:], rs[:])
            # M = max(M, e*rinv) via fast (non-aliased) fp16 tensor ops
            nc.vector.tensor_scalar(rinv512[:], ones512[:], rinv[:], None, ALU.mult)
            src = macc[it % 2]
            dst_m = macc[(it + 1) % 2]
            for q in range(NCH):
                cols = slice(q * CH, (q + 1) * CH)
                nc.vector.tensor_mul(em[:, cols], e16[:, cols], rinv512[:])
                nc.vector.tensor_max(dst_m[:, cols], src[:, cols], em[:, cols])

        nc.sync.dma_start(m_dram[:, :], macc[len(ib_list) % 2][:])
    nc.compile()
    return nc


_NC_CACHE = {}


def _get_nc():
    if "nc" not in _NC_CACHE:
        _NC_CACHE["nc"] = _build_nc()
    return _NC_CACHE["nc"]


def kernel(pred, target, _trace=False):
    pred = np.asarray(pred, dtype=np.float32).reshape(N_IMG, C, HW)
    target = np.asarray(target, dtype=np.float32).reshape(N_IMG, C, HW)
    nc = _get_nc()
    in_maps = []
    for core in range(8):
        img, half = divmod(core, 2)
        in_maps.append({
            "t": np.ascontiguousarray(target[img]),
            "p": np.ascontiguousarray(pred[img, :, half * R:(half + 1) * R]),
        })
    res = run_bass_kernel_spmd(nc, in_maps, list(range(8)), trace=_trace)
    losses = []
    for img in range(N_IMG):
        m0 = res.results[2 * img]["m_out"].astype(np.float32).max(axis=0)
        m1 = res.results[2 * img + 1]["m_out"].astype(np.float32).max(axis=0)
        cx = np.maximum(m0, m1).mean()
        losses.append(-np.log(cx + EPS))
    out = np.float32(np.mean(losses))
    if _trace:
        return out, res
    return out


# revision 14
# speedup vs baseline: 1.1676x; 1.1644x over previous
"""Contextual loss (CX) kernel for Trainium2, 8 NeuronCores.

Sharding: data-parallel over (image, row-half): core c handles image c//2,
pred-rows [ (c%2)*2048, (c%2+1)*2048 ) of the 4096x4096 contextual matrix.

Math (per core, rows i of its half, columns j over all HW):
    pc_i  = p_i - mu        (mu = target mean feature; fp8 quantized)
    tc_j  = t_j - mu        (fp8 quantized, NOT normalized)
    raw_ij = <pc_i, tc_j>   (fp8 DoubleRow matmul, fp32 PSUM)
    s_ij  = raw_ij * invm_j (fused into the PSUM eviction; invm = 1/||tc||)
    e_ij  = exp(b_i q_i (s_ij - smax_i)) with q_i = 1/||pc_i||,
            b_i = 1/(1+EPS-q_i*smax_i)   (per-row scale/bias inside ACT exp)
    rs_i  = sum_j e_ij      (ACT accumulate)
    M_j   = max_i e_ij / rs_i  (partition-wise partial column max)
Host folds partitions + row-halves, means over j, -log, means over N.
Identical to the reference up to per-row constants that cancel in the
row normalization.

Engine split per 128-row block (PSUM held as four 2-bank pair tiles):
  PE   16 fp8 DoubleRow matmuls
  DVE  4 tensor_tensor_reduce evictions (x invm, fused row-max accum);
       per-row scalar chain; deferred ping-pong column-max folds
  ACT  exp with rowsum-accumulate; em = e * rinv via Copy-with-scale
"""

import os
import numpy as np
from contextlib import ExitStack

import concourse.bass as bass
import concourse.bacc as bacc
import concourse.mybir as mybir
import concourse.tile as tile
from concourse.bass_utils import run_bass_kernel_spmd

F32 = mybir.dt.float32
F16 = mybir.dt.float16
F8 = mybir.dt.float8e4
AX = mybir.AxisListType.X
ALU = mybir.AluOpType
ACTF = mybir.ActivationFunctionType
DR = mybir.MatmulPerfMode.DoubleRow

N_IMG, C, H, W = 4, 512, 64, 64
HW = H * W              # 4096
R = HW // 2             # 2048 rows per core
KB = C // 128           # 4 contraction blocks
NPAIR = KB // 2         # 2 DoubleRow pairs
NB = R // 128           # 16 row blocks per core
CH = 512                # one PSUM bank
NCH = HW // CH          # 8 chunks
PW = 2 * CH             # PSUM pair-tile width
EPS = 1e-5
NEG_BIG = -1e30


def _build_nc():
    nc = bacc.Bacc("TRN2", target_bir_lowering=False, debug=False, num_devices=8)
    t_dram = nc.dram_tensor("t", [C, HW], F32, kind="ExternalInput").ap()
    p_dram = nc.dram_tensor("p", [C, R], F32, kind="ExternalInput").ap()
    m_dram = nc.dram_tensor("m_out", [128, HW], F16, kind="ExternalOutput").ap()

    with tile.TileContext(nc) as tc, ExitStack() as ctx:
        const = ctx.enter_context(tc.tile_pool(name="const", bufs=1))
        ones16 = const.tile([128, 128], F16, tag="ones", name="ones16")
        nc.vector.memset(ones16[:], 1.0)
        # fp8 operands in DoubleRow pair-interleaved layout: pair p holds
        # contraction blocks 2p (dim1=0) and 2p+1 (dim1=1)
        tc8 = [const.tile([128, 2, HW], F8, tag=f"tc8_{p}", name=f"tc8_{p}")
               for p in range(NPAIR)]
        pc8 = [const.tile([128, 2, R], F8, tag=f"pc{p}", name=f"pc{p}")
               for p in range(NPAIR)]
        rinvn = const.tile([128, NB], F32, tag="rinvn", name="rinvn")
        invm = const.tile([128, HW], F16, tag="invm", name="invm")

        # ---------------- preprocessing ----------------
        with (
            tc.tile_pool(name="raw", bufs=1) as raw,
            tc.tile_pool(name="sqp", bufs=2) as sqp,
        ):
            traw = [raw.tile([128, HW], F32, tag=f"traw{k}", name=f"traw{k}") for k in range(KB)]
            praw = [raw.tile([128, R], F32, tag=f"praw{k}", name=f"praw{k}") for k in range(KB)]
            t16 = [raw.tile([128, HW], F16, tag=f"t16_{k}", name=f"t16_{k}") for k in range(KB)]
            tsum = [raw.tile([128, 1], F32, tag=f"tsum{k}", name=f"tsum{k}") for k in range(KB)]
            negmu = [raw.tile([128, 1], F32, tag=f"negmu{k}", name=f"negmu{k}") for k in range(KB)]
            psq = [raw.tile([128, R], F16, tag=f"psq{k}", name=f"psq{k}") for k in range(KB)]
            lnm = raw.tile([128, HW], F16, tag="lnm", name="lnm")
            nsq_sb = raw.tile([128, NB], F32, tag="nsq_sb", name="nsq_sb")
            lnn = raw.tile([128, NB], F32, tag="lnn", name="lnn")

            for k in range(KB):
                nc.sync.dma_start(traw[k][:], t_dram[k * 128:(k + 1) * 128, :])
            for k in range(KB):
                nc.sync.dma_start(praw[k][:], p_dram[k * 128:(k + 1) * 128, :])

            # target per-channel mean (ACT accumulate) + f16 cast
            for k in range(KB):
                nc.scalar.activation(t16[k][:], traw[k][:], ACTF.Identity,
                                     accum_out=tsum[k][:])
            for k in range(KB):
                nc.vector.tensor_scalar(negmu[k][:], tsum[k][:], -1.0 / HW, None, ALU.mult)

            # pred: center -> fp8 (ACT, fused bias); square from the
            # quantized pc; transposed column-sums -> nsq [128, NB]
            for k in range(KB):
                pslice = pc8[k // 2][:, k % 2, :]
                nc.scalar.activation(pslice, praw[k][:], ACTF.Identity,
                                     bias=negmu[k][:])
                nc.vector.tensor_mul(psq[k][:], pslice, pslice)
            with tc.tile_pool(name="nsqps", bufs=1, space="PSUM") as nsqps:
                nsq_ps = nsqps.tile([128, NB], F32, tag="nsq", name="nsq_ps")
                for ib in range(NB):
                    for k in range(KB):
                        nc.tensor.matmul(
                            nsq_ps[:, ib:ib + 1],
                            psq[k][:, ib * 128:(ib + 1) * 128],
                            ones16[:, 0:1],
                            start=(k == 0),
                            stop=(k == KB - 1),
                        )
                nc.vector.tensor_scalar(nsq_sb[:], nsq_ps[:], 1.0, None, ALU.mult)

            # target: center in place (f16), square, column-sums -> msq
            with tc.tile_pool(name="msqps", bufs=1, space="PSUM") as msqps:
                msq = msqps.tile([128, HW], F32, tag="msq", name="msq")
                for k in range(KB):
                    nc.vector.tensor_scalar(t16[k][:], t16[k][:], negmu[k][:], None, ALU.add)
                    sq = sqp.tile([128, HW], F16, tag="sq", name="sq")
                    nc.vector.tensor_mul(sq[:], t16[k][:], t16[k][:])
                    for j in range(NCH):
                        nc.tensor.matmul(
                            msq[:, j * CH:(j + 1) * CH],
                            ones16[:],
                            sq[:, j * CH:(j + 1) * CH],
                            start=(k == 0),
                            stop=(k == KB - 1),
                        )

                # x^-0.5 = exp(-0.5 ln x)
                nc.scalar.activation(lnn[:], nsq_sb[:], ACTF.Ln)
                nc.scalar.activation(lnm[:], msq[:], ACTF.Ln)
            nc.scalar.activation(invm[:], lnm[:], ACTF.Exp, scale=-0.5)
            nc.scalar.activation(rinvn[:], lnn[:], ACTF.Exp, scale=-0.5)

            # that8 = (t - mu) * invm -> fp8 (normalized target operand)
            for k in range(KB):
                nc.vector.tensor_mul(tc8[k // 2][:, k % 2, :], t16[k][:], invm[:])

        # ---------------- main loop ----------------
        main = ctx.enter_context(tc.tile_pool(name="main", bufs=2))
        stats = ctx.enter_context(tc.tile_pool(name="stats", bufs=2))
        mainps = ctx.enter_context(tc.tile_pool(name="mainps", bufs=4, space="PSUM"))
        # ping-pong column-max accumulators (DVE tensor_max needs out to not
        # alias its inputs for full speed)
        macc = [main.tile([128, HW], F16, tag=f"mACC{i}", bufs=1, name=f"mACC{i}")
                for i in range(2)]
        nc.vector.memset(macc[0][:], 0.0)

        reps = int(os.environ.get("CX_REPS", "1"))
        ib_list = [i for _ in range(reps) for i in range(NB)]
        pending = None  # deferred macc fold from the previous block
        for it, ib in enumerate(ib_list):
            s16 = main.tile([128, HW], F16, tag="s", name="s16")
            e16 = main.tile([128, HW], F16, tag="e", name="e16")
            em = main.tile([128, HW], F16, tag="em", name="em")
            cmax = stats.tile([128, 4], F32, tag="cmax", name="cmax")
            rawmax = stats.tile([128, 1], F32, tag="rawmax", name="rawmax")
            smax = stats.tile([128, 1], F32, tag="smax", name="smax")
            t1 = stats.tile([128, 1], F32, tag="t1", name="t1")
            bb = stats.tile([128, 1], F32, tag="bb", name="bb")
            scaleP = stats.tile([128, 1], F32, tag="scaleP", name="scaleP")
            biasP = stats.tile([128, 1], F32, tag="biasP", name="biasP")
            rs = stats.tile([128, 1], F32, tag="rs", name="rs")
            rinv = stats.tile([128, 1], F32, tag="rinv", name="rinv")
            qv = rinvn[:, ib:ib + 1]

            # four 2-bank PSUM pair tiles; chunks 2p, 2p+1 live in pair p
            pss = [mainps.tile([128, PW], F32, tag="ps", name="ps") for _ in range(4)]
            for jc in range(NCH):
                pt = pss[jc // 2]
                out = pt[:, (jc % 2) * CH:(jc % 2 + 1) * CH]
                for pair in range(NPAIR):
                    nc.tensor.matmul(
                        out,
                        pc8[pair][:, :, ib * 128:(ib + 1) * 128],
                        tc8[pair][:, :, jc * CH:(jc + 1) * CH],
                        start=(pair == 0),
                        stop=(pair == NPAIR - 1),
                        perf_mode=DR,
                    )
                if jc % 2 == 1:
                    # fused PSUM->SBUF eviction + row-max accumulation
                    p = jc // 2
                    cols = slice(p * PW, (p + 1) * PW)
                    nc.vector.tensor_scalar(
                        s16[:, cols], pt[:], 1.0, None, ALU.mult, ALU.max,
                        accum_out=cmax[:, p:p + 1],
                    )

            # deferred column-max fold for the previous block
            if pending is not None:
                em_p, src = pending
                for half in range(2):
                    cols = slice(half * (HW // 2), (half + 1) * (HW // 2))
                    nc.vector.tensor_max(macc[1 - src][:, cols],
                                         macc[src][:, cols], em_p[:, cols])

            nc.vector.reduce_max(rawmax[:], cmax[:], axis=AX)
            # b=1/(1+EPS-rawmax*q); scale=b*q; bias=-scale*rawmax
            nc.vector.tensor_mul(smax[:], rawmax[:], qv)
            nc.vector.tensor_scalar(t1[:], smax[:], -1.0, 1.0 + EPS, ALU.mult, ALU.add)
            nc.vector.reciprocal(bb[:], t1[:])
            nc.vector.tensor_mul(scaleP[:], bb[:], qv)
            nc.vector.scalar_tensor_tensor(
                biasP[:], scaleP[:], -1.0, rawmax[:], ALU.mult, ALU.mult
            )

            nc.scalar.activation(
                e16[:], s16[:], ACTF.Exp, bias=biasP[:], scale=scaleP[:],
                accum_out=rs[:],
            )
            nc.vector.reciprocal(rinv[:], rs[:])
            # em = e * rinv on ACT (Identity with per-partition scale)
            nc.scalar.activation(em[:], e16[:], ACTF.Identity, scale=rinv[:])
            pending = (em, it % 2)

        em_p, src = pending
        for half in range(2):
            cols = slice(half * (HW // 2), (half + 1) * (HW // 2))
            nc.vector.tensor_max(macc[1 - src][:, cols],
                                 macc[src][:, cols], em_p[:, cols])
        nc.sync.dma_start(m_dram[:, :], macc[len(ib_list) % 2][:])
    nc.compile()
    return nc


_NC_CACHE = {}


def _get_nc():
    if "nc" not in _NC_CACHE:
        _NC_CACHE["nc"] = _build_nc()
    return _NC_CACHE["nc"]


def kernel(pred, target, _trace=False):
    pred = np.asarray(pred, dtype=np.float32).reshape(N_IMG, C, HW)
    target = np.asarray(target, dtype=np.float32).reshape(N_IMG, C, HW)
    nc = _get_nc()
    in_maps = []
    for core in range(8):
        img, half = divmod(core, 2)
        in_maps.append({
            "t": np.ascontiguousarray(target[img]),
            "p": np.ascontiguousarray(pred[img, :, half * R:(half + 1) * R]),
        })
    res = run_bass_kernel_spmd(nc, in_maps, list(range(8)), trace=_trace)
    losses = []
    for img in range(N_IMG):
        m0 = res.results[2 * img]["m_out"].astype(np.float32).max(axis=0)
        m1 = res.results[2 * img + 1]["m_out"].astype(np.float32).max(axis=0)
        cx = np.maximum(m0, m1).mean()
        losses.append(-np.log(cx + EPS))
    out = np.float32(np.mean(losses))
    if _trace:
        return out, res
    return out


# revision 16
# speedup vs baseline: 1.2117x; 1.0378x over previous
"""Contextual loss (CX) kernel for Trainium2, 8 NeuronCores.

Sharding: data-parallel over (image, row-half): core c handles image c//2,
pred-rows [ (c%2)*2048, (c%2+1)*2048 ) of the 4096x4096 contextual matrix.

Math (per core, rows i of its half, columns j over all HW):
    pc_i   = p_i - mu          (mu = target mean feature; fp8 quantized)
    that_j = (t_j - mu)/||t_j - mu||                      (fp8 quantized)
    raw_ij = <pc_i, that_j>    (fp8 DoubleRow matmul, fp32 PSUM)
    s_ij   = raw_ij / n_i,  n_i = ||pc_i||  (from the quantized pc)
    e_ij   = exp(b_i (s_ij - smax_i)) = exp(scale_i*raw_ij + bias_i)
    rs_i   = sum_j e_ij        (ACT accumulate)
    M_j    = max(M_j, e_ij / rs_i)   (partition-wise partial column max)
Host folds partitions + row-halves, means over j, -log, means over N.
Identical to the reference up to per-row constants that cancel in the
row normalization.

Steady-state pipeline per 128-row block (period ~= exp+em on ACT):
  PE   16 fp8 DoubleRow matmuls into four 2-bank PSUM pair tiles
  DVE  4 fused evictions (PSUM->f16 + row-max accum), scalar chain,
       then (deferred) ping-pong column-max folds of block it-2 and the
       rowsum reciprocal of block it-1
  ACT  exp(it) with rowsum accumulate, then em(it-1) = e*rinv via
       Identity-with-scale
"""

import os
import numpy as np
from contextlib import ExitStack

import concourse.bass as bass
import concourse.bacc as bacc
import concourse.mybir as mybir
import concourse.tile as tile
from concourse.bass_utils import run_bass_kernel_spmd

F32 = mybir.dt.float32
F16 = mybir.dt.float16
F8 = mybir.dt.float8e4
AX = mybir.AxisListType.X
ALU = mybir.AluOpType
ACTF = mybir.ActivationFunctionType
DR = mybir.MatmulPerfMode.DoubleRow

N_IMG, C, H, W = 4, 512, 64, 64
HW = H * W              # 4096
R = HW // 2             # 2048 rows per core
KB = C // 128           # 4 contraction blocks
NPAIR = KB // 2         # 2 DoubleRow pairs
NB = R // 128           # 16 row blocks per core
CH = 512                # one PSUM bank
NCH = HW // CH          # 8 chunks
PW = 2 * CH             # PSUM pair-tile width
HH = HW // 2
EPS = 1e-5


def _build_nc():
    nc = bacc.Bacc("TRN2", target_bir_lowering=False, debug=False, num_devices=8)
    t_dram = nc.dram_tensor("t", [C, HW], F32, kind="ExternalInput").ap()
    p_dram = nc.dram_tensor("p", [C, R], F32, kind="ExternalInput").ap()
    m_dram = nc.dram_tensor("m_out", [128, HW], F16, kind="ExternalOutput").ap()

    with tile.TileContext(nc) as tc, ExitStack() as ctx:
        const = ctx.enter_context(tc.tile_pool(name="const", bufs=1))
        ones16 = const.tile([128, 128], F16, tag="ones", name="ones16")
        nc.vector.memset(ones16[:], 1.0)
        # fp8 operands in DoubleRow pair-interleaved layout: pair p holds
        # contraction blocks 2p (dim1=0) and 2p+1 (dim1=1)
        that8 = [const.tile([128, 2, HW], F8, tag=f"that{p}", name=f"that{p}")
                 for p in range(NPAIR)]
        pc8 = [const.tile([128, 2, R], F8, tag=f"pc{p}", name=f"pc{p}")
               for p in range(NPAIR)]
        rinvn = const.tile([128, NB], F32, tag="rinvn", name="rinvn")

        # ---------------- preprocessing ----------------
        with (
            tc.tile_pool(name="raw", bufs=1) as raw,
            tc.tile_pool(name="sqp", bufs=2) as sqp,
        ):
            traw = [raw.tile([128, HW], F32, tag=f"traw{k}", name=f"traw{k}") for k in range(KB)]
            praw = [raw.tile([128, R], F32, tag=f"praw{k}", name=f"praw{k}") for k in range(KB)]
            tsum = [raw.tile([128, 1], F32, tag=f"tsum{k}", name=f"tsum{k}") for k in range(KB)]
            negmu = [raw.tile([128, 1], F32, tag=f"negmu{k}", name=f"negmu{k}") for k in range(KB)]
            psq = [raw.tile([128, R], F16, tag=f"psq{k}", name=f"psq{k}") for k in range(KB)]
            junk = raw.tile([128, HW], F16, tag="junk", name="junk")
            lnm = raw.tile([128, HW], F16, tag="lnm", name="lnm")
            invm = raw.tile([128, HW], F16, tag="invm", name="invm")
            nsq_sb = raw.tile([128, NB], F32, tag="nsq_sb", name="nsq_sb")
            lnn = raw.tile([128, NB], F32, tag="lnn", name="lnn")

            for k in range(KB):
                nc.sync.dma_start(traw[k][:], t_dram[k * 128:(k + 1) * 128, :])
            for k in range(KB):
                nc.sync.dma_start(praw[k][:], p_dram[k * 128:(k + 1) * 128, :])

            # target per-channel mean (ACT accumulate; output unused)
            for k in range(KB):
                nc.scalar.activation(junk[:], traw[k][:], ACTF.Identity,
                                     accum_out=tsum[k][:])
            for k in range(KB):
                nc.vector.tensor_scalar(negmu[k][:], tsum[k][:], -1.0 / HW, None, ALU.mult)

            # pred: center -> fp8 (DVE); square from the quantized pc;
            # transposed column-sums -> nsq [128, NB]
            for k in range(KB):
                pslice = pc8[k // 2][:, k % 2, :]
                nc.vector.tensor_scalar(pslice, praw[k][:], negmu[k][:], None, ALU.add)
                nc.vector.tensor_mul(psq[k][:], pslice, pslice)
            with tc.tile_pool(name="nsqps", bufs=1, space="PSUM") as nsqps:
                nsq_ps = nsqps.tile([128, NB], F32, tag="nsq", name="nsq_ps")
                for ib in range(NB):
                    for k in range(KB):
                        nc.tensor.matmul(
                            nsq_ps[:, ib:ib + 1],
                            psq[k][:, ib * 128:(ib + 1) * 128],
                            ones16[:, 0:1],
                            start=(k == 0),
                            stop=(k == KB - 1),
                        )
                nc.vector.tensor_scalar(nsq_sb[:], nsq_ps[:], 1.0, None, ALU.mult)

            # target: fused center+square on ACT, column-sums -> msq
            with tc.tile_pool(name="msqps", bufs=1, space="PSUM") as msqps:
                msq = msqps.tile([128, HW], F32, tag="msq", name="msq")
                for k in range(KB):
                    sq = sqp.tile([128, HW], F16, tag="sq", name="sq")
                    nc.scalar.activation(sq[:], traw[k][:], ACTF.Square,
                                         bias=negmu[k][:])
                    for j in range(NCH):
                        nc.tensor.matmul(
                            msq[:, j * CH:(j + 1) * CH],
                            ones16[:],
                            sq[:, j * CH:(j + 1) * CH],
                            start=(k == 0),
                            stop=(k == KB - 1),
                        )

                # x^-0.5 = exp(-0.5 ln x); halves so that8 can start early
                nc.scalar.activation(lnn[:], nsq_sb[:], ACTF.Ln)
                nc.scalar.activation(lnm[:], msq[:], ACTF.Ln)
            nc.scalar.activation(rinvn[:], lnn[:], ACTF.Exp, scale=-0.5)
            for h in range(2):
                cols = slice(h * HH, (h + 1) * HH)
                nc.scalar.activation(invm[:, cols], lnm[:, cols], ACTF.Exp, scale=-0.5)
                # that8 = (t - mu) * invm -> fp8, fused center+normalize+cast
                for k in range(KB):
                    nc.vector.scalar_tensor_tensor(
                        that8[k // 2][:, k % 2, cols], traw[k][:, cols],
                        negmu[k][:], invm[:, cols], ALU.add, ALU.mult,
                    )

        # ---------------- main loop ----------------
        main = ctx.enter_context(tc.tile_pool(name="main", bufs=2))
        stats = ctx.enter_context(tc.tile_pool(name="stats", bufs=2))
        mainps = ctx.enter_context(tc.tile_pool(name="mainps", bufs=4, space="PSUM"))
        # ping-pong column-max accumulators (tensor_max out must not alias)
        macc = [main.tile([128, HW], F16, tag=f"mACC{i}", bufs=1, name=f"mACC{i}")
                for i in range(2)]
        nc.vector.memset(macc[0][:], 0.0)

        reps = int(os.environ.get("CX_REPS", "1"))
        ib_list = [i for _ in range(reps) for i in range(NB)]
        N = len(ib_list)
        # per-iteration live state for the software pipeline
        e_t = [None] * N
        em_t = [None] * N
        rs_t = [None] * N
        rinv_t = [None] * N

        def fold_maxes(j):
            em_p = em_t[j]
            for half in range(2):
                cols = slice(half * HH, (half + 1) * HH)
                nc.vector.tensor_max(macc[(j + 1) % 2][:, cols],
                                     macc[j % 2][:, cols], em_p[:, cols])

        def do_em(j):
            em = main.tile([128, HW], F16, tag="em", bufs=3, name="em")
            nc.scalar.activation(em[:], e_t[j][:], ACTF.Identity, scale=rinv_t[j][:])
            em_t[j] = em

        def do_rinv(j):
            rv = stats.tile([128, 1], F32, tag="rinv", name="rinv")
            nc.vector.reciprocal(rv[:], rs_t[j][:])
            rinv_t[j] = rv

        for it, ib in enumerate(ib_list):
            s16 = main.tile([128, HW], F16, tag="s", name="s16")
            cmax = stats.tile([128, 4], F32, tag="cmax", name="cmax")
            rawmax = stats.tile([128, 1], F32, tag="rawmax", name="rawmax")
            smax = stats.tile([128, 1], F32, tag="smax", name="smax")
            t1 = stats.tile([128, 1], F32, tag="t1", name="t1")
            bb = stats.tile([128, 1], F32, tag="bb", name="bb")
            scaleP = stats.tile([128, 1], F32, tag="scaleP", name="scaleP")
            biasP = stats.tile([128, 1], F32, tag="biasP", name="biasP")
            qv = rinvn[:, ib:ib + 1]

            # four 2-bank PSUM pair tiles; chunks 2p, 2p+1 live in pair p
            pss = [mainps.tile([128, PW], F32, tag="ps", name="ps") for _ in range(4)]
            for jc in range(NCH):
                pt = pss[jc // 2]
                out = pt[:, (jc % 2) * CH:(jc % 2 + 1) * CH]
                for pair in range(NPAIR):
                    nc.tensor.matmul(
                        out,
                        pc8[pair][:, :, ib * 128:(ib + 1) * 128],
                        that8[pair][:, :, jc * CH:(jc + 1) * CH],
                        start=(pair == 0),
                        stop=(pair == NPAIR - 1),
                        perf_mode=DR,
                    )
                if jc % 2 == 1:
                    # fused PSUM->SBUF eviction + row-max accumulation
                    p = jc // 2
                    cols = slice(p * PW, (p + 1) * PW)
                    nc.vector.tensor_scalar(
                        s16[:, cols], pt[:], 1.0, None, ALU.mult, ALU.max,
                        accum_out=cmax[:, p:p + 1],
                    )

            nc.vector.reduce_max(rawmax[:], cmax[:], axis=AX)
            # b=1/(1+EPS-rawmax*q); scale=b*q; bias=-scale*rawmax
            nc.vector.tensor_mul(smax[:], rawmax[:], qv)
            nc.vector.tensor_scalar(t1[:], smax[:], -1.0, 1.0 + EPS, ALU.mult, ALU.add)
            nc.vector.reciprocal(bb[:], t1[:])
            nc.vector.tensor_mul(scaleP[:], bb[:], qv)
            nc.vector.scalar_tensor_tensor(
                biasP[:], scaleP[:], -1.0, rawmax[:], ALU.mult, ALU.mult
            )

            # DVE tail (deferred work, no stalls): reciprocal for it-1,
            # column-max folds for it-2
            if it >= 1:
                do_rinv(it - 1)
            if it >= 2:
                fold_maxes(it - 2)

            e16 = main.tile([128, HW], F16, tag="e", name="e16")
            rs = stats.tile([128, 1], F32, tag="rs", name="rs")
            nc.scalar.activation(
                e16[:], s16[:], ACTF.Exp, bias=biasP[:], scale=scaleP[:],
                accum_out=rs[:],
            )
            e_t[it] = e16
            rs_t[it] = rs

            # ACT: em for the previous block (after this block's exp)
            if it >= 1:
                do_em(it - 1)

        # drain the pipeline
        do_rinv(N - 1)
        do_em(N - 1)
        fold_maxes(N - 2)
        fold_maxes(N - 1)
        nc.sync.dma_start(m_dram[:, :], macc[N % 2][:])
    nc.compile()
    return nc


_NC_CACHE = {}


def _get_nc():
    if "nc" not in _NC_CACHE:
        _NC_CACHE["nc"] = _build_nc()
    return _NC_CACHE["nc"]


def kernel(pred, target, _trace=False):
    pred = np.asarray(pred, dtype=np.float32).reshape(N_IMG, C, HW)
    target = np.asarray(target, dtype=np.float32).reshape(N_IMG, C, HW)
    nc = _get_nc()
    in_maps = []
    for core in range(8):
        img, half = divmod(core, 2)
        in_maps.append({
            "t": np.ascontiguousarray(target[img]),
            "p": np.ascontiguousarray(pred[img, :, half * R:(half + 1) * R]),
        })
    res = run_bass_kernel_spmd(nc, in_maps, list(range(8)), trace=_trace)
    losses = []
    for img in range(N_IMG):
        m0 = res.results[2 * img]["m_out"].astype(np.float32).max(axis=0)
        m1 = res.results[2 * img + 1]["m_out"].astype(np.float32).max(axis=0)
        cx = np.maximum(m0, m1).mean()
        losses.append(-np.log(cx + EPS))
    out = np.float32(np.mean(losses))
    if _trace:
        return out, res
    return out
